# revision 9
# baseline (speedup 1.0000x reference)
"""BasisResidualFFN Trainium2 kernel.

Math (per token t):
  recipe_soft = softmax(neuron_recipe, axis=-1)                 [64, 16]
  tr[t, :]    = sum_k w[t,k] * recipe_soft[idx[t,k], :]         [16]
  Y[t, (n,r)] = sum_d x[t,d] * basis_A[n,d,r]
  h[t, r]     = sum_n tr[t,n] * Y[t,(n,r)]
  delta[t, d] = sum_{n,r} basis_A[n,d,r] * tr[t,n] * h[t,r]
  out         = gelu((x + alpha*delta) @ w_up + b_up) @ w_down + b_down

Distribution: pure data parallel. B*S = 4096 tokens sharded 512/core
across 8 NeuronCores; all weights replicated. Everything on device is
computed feature-major (features on partitions, tokens on the free
axis, 512 tokens per matmul) so no on-device activation transposes are
needed anywhere in the FFN; x arrives pre-transposed from the host and
the output is un-transposed on the host. Matmuls run in float32r
(full-rate fp32, tf32-like precision). alpha is folded into basis_A's
second copy on device (exact, by linearity).
"""

import numpy as np

import concourse.bass as bass
import concourse.mybir as mybir
import concourse.tile as tile
from concourse import bacc
from concourse.bass import ts
from concourse.bass_utils import run_bass_kernel_spmd

P = 128
NCORES = 8
T = 512            # tokens per core
D = 1024
DFF = 4096
NB = 16            # n_basis
R = 32             # rank
NN = 64            # n_neurons
K = 8              # top-k
DC = D // P        # 8 contraction chunks over d
FT = DFF // P      # 32 ff tiles
DT = D // P        # 8 output d tiles
NRT = (NB * R) // P  # 4 (n,r) tiles
TT = T // P        # 4 token tiles per core

F32 = mybir.dt.float32
F32R = mybir.dt.float32r

_BUILT = [None]


def _build_nc():
    nc = bacc.Bacc(None, target_bir_lowering=False)

    xt_d = nc.dram_tensor("xt", [P, DC, T], F32R, kind="ExternalInput")
    idx_d = nc.dram_tensor("idxf", [T, K], F32, kind="ExternalInput")
    wgt_d = nc.dram_tensor("wgt", [T, K], F32, kind="ExternalInput")
    rec_d = nc.dram_tensor("recipe", [NN, NB], F32, kind="ExternalInput")
    a1_d = nc.dram_tensor("a1", [P, DC, NB * R], F32R, kind="ExternalInput")
    a2_d = nc.dram_tensor("a2", [P, NRT, D], F32R, kind="ExternalInput")
    wu_d = nc.dram_tensor("wu", [FT, P, DC, P], F32R, kind="ExternalInput")
    wd_d = nc.dram_tensor("wd", [DT, P, FT, P], F32R, kind="ExternalInput")
    bu_d = nc.dram_tensor("bu", [P, FT], F32, kind="ExternalInput")
    bd_d = nc.dram_tensor("bd", [P, DT], F32, kind="ExternalInput")
    al_d = nc.dram_tensor("alphac", [P, 1], F32, kind="ExternalInput")
    sel_d = nc.dram_tensor("selpat", [NB, NRT, P], F32R, kind="ExternalInput")
    trep_d = nc.dram_tensor("trep", [R, P], F32R, kind="ExternalInput")
    qred_d = nc.dram_tensor("qred", [P, R], F32R, kind="ExternalInput")
    iota_d = nc.dram_tensor("iota64", [P, NN], F32, kind="ExternalInput")
    ident_d = nc.dram_tensor("ident", [P, P], F32, kind="ExternalInput")
    out_d = nc.dram_tensor("outT", [P, DT, T], F32, kind="ExternalOutput")

    AX = mybir.AxisListType.X
    AF = mybir.ActivationFunctionType
    ALU = mybir.AluOpType

    with tile.TileContext(nc) as tc:
        with (
            tc.tile_pool(name="const", bufs=1) as constp,
            tc.tile_pool(name="stream", bufs=3) as stream,
            tc.tile_pool(name="wdstream", bufs=4) as wdstream,
            tc.tile_pool(name="mid", bufs=1) as mid,
            tc.tile_pool(name="small", bufs=2) as small,
            tc.tile_pool(name="psum", bufs=4, space="PSUM") as psum,
            tc.tile_pool(name="psums", bufs=1, space="PSUM") as psums,
        ):
            # ---- resident constants / activations ----
            xt = [constp.tile([P, T], F32R, tag=f"xt{i}", name=f"xt{i}") for i in range(DC)]
            for i in range(DC):
                nc.sync.dma_start(xt[i][:], xt_d[:, i, :])
            a1 = [constp.tile([P, NB * R], F32R, tag=f"a1_{i}", name=f"a1_{i}") for i in range(DC)]
            for i in range(DC):
                nc.sync.dma_start(a1[i][:], a1_d[:, i, :])
            a2 = [constp.tile([P, D], F32R, tag=f"a2_{i}", name=f"a2_{i}") for i in range(NRT)]
            for i in range(NRT):
                nc.sync.dma_start(a2[i][:], a2_d[:, i, :])
            sel = constp.tile([NB, NRT, P], F32R, tag="sel")
            nc.sync.dma_start(sel[:], sel_d[:])
            trep = constp.tile([R, P], F32R, tag="trep")
            nc.sync.dma_start(trep[:], trep_d[:])
            qred = constp.tile([P, R], F32R, tag="qred")
            nc.sync.dma_start(qred[:], qred_d[:])
            iota = constp.tile([P, NN], F32, tag="iota")
            nc.sync.dma_start(iota[:], iota_d[:])
            ident = constp.tile([P, P], F32, tag="ident")
            nc.sync.dma_start(ident[:], ident_d[:])
            bu = constp.tile([P, FT], F32, tag="bu")
            nc.sync.dma_start(bu[:], bu_d[:])
            bd = constp.tile([P, DT], F32, tag="bd")
            nc.sync.dma_start(bd[:], bd_d[:])
            alpha = constp.tile([P, 1], F32, tag="alpha")
            nc.sync.dma_start(alpha[:], al_d[:])
            rec = constp.tile([NN, NB], F32, tag="rec")
            nc.sync.dma_start(rec[:], rec_d[:])

            # fold alpha into A2 (delta path) so x_f = x + deltaT directly
            for i in range(NRT):
                nc.vector.tensor_scalar_mul(a2[i][:], a2[i][:], alpha[:, 0:1])

            # ---- softmax over the 16-basis axis of the recipe table ----
            mx = small.tile([NN, 1], F32, tag="mx")
            nc.vector.reduce_max(mx[:], rec[:], axis=AX)
            negmx = small.tile([NN, 1], F32, tag="negmx")
            nc.vector.tensor_scalar_mul(negmx[:], mx[:], -1.0)
            esb = small.tile([NN, NB], F32, tag="esb")
            nc.scalar.activation(esb[:], rec[:], AF.Exp, bias=negmx[:, 0:1], scale=1.0)
            ssum = small.tile([NN, 1], F32, tag="ssum")
            nc.vector.reduce_sum(ssum[:], esb[:], axis=AX)
            rsum = small.tile([NN, 1], F32, tag="rsum")
            nc.vector.reciprocal(rsum[:], ssum[:])
            recs = constp.tile([NN, NB], F32R, tag="recs")
            nc.vector.tensor_scalar_mul(recs[:], esb[:], rsum[:, 0:1])

            # ---- routing: weighted one-hot scatter S[t, neuron], transposed ----
            st_sb = constp.tile([NN, T], F32R, tag="st")
            for tt in range(TT):
                idxt = small.tile([P, K], F32, tag="idxt")
                wt = small.tile([P, K], F32, tag="wt")
                nc.sync.dma_start(idxt[:], idx_d[ts(tt, P), :])
                nc.sync.dma_start(wt[:], wgt_d[ts(tt, P), :])
                sk = small.tile([P, NN, K], F32, tag="sk")
                for k in range(K):
                    nc.vector.tensor_scalar(
                        sk[:, :, k], iota[:],
                        idxt[:, k:k + 1], wt[:, k:k + 1],
                        ALU.is_equal, ALU.mult,
                    )
                s_tile = small.tile([P, NN], F32, tag="s")
                nc.vector.reduce_sum(s_tile[:], sk[:], axis=AX)
                stp = psums.tile([NN, P], F32, tag="stp")
                nc.tensor.transpose(stp[:], s_tile[:], ident[:])
                nc.any.tensor_copy(st_sb[:, ts(tt, P)], stp[:])

            # token recipes, transposed: recipeT[n, t]
            rt_ps = psums.tile([NB, T], F32, tag="rtps")
            nc.tensor.matmul(rt_ps[:], recs[:], st_sb[:], start=True, stop=True)
            recipeT = constp.tile([NB, T], F32R, tag="recipeT")
            nc.any.tensor_copy(recipeT[:], rt_ps[:])

            # RepR[(n,r), t] = recipeT[n, t] replicated over r (per nr-tile)
            repr_sb = []
            for i in range(NRT):
                rp = psum.tile([P, T], F32, tag="ps")
                nc.tensor.matmul(rp[:], sel[:, i, :], recipeT[:], start=True, stop=True)
                rr = constp.tile([P, T], F32, tag=f"repr{i}", name=f"repr{i}")
                nc.any.tensor_copy(rr[:], rp[:])
                repr_sb.append(rr)

            # ---- YT = A1^T @ xT;  WYT = YT * RepR;  hT = sum_n WYT ----
            ht_ps = psums.tile([R, T], F32, tag="htps")
            wyt = [mid.tile([P, T], F32R, tag=f"mid{i}", name=f"wyt{i}") for i in range(NRT)]
            for i in range(NRT):
                yt_ps = psum.tile([P, T], F32, tag="ps")
                for dc in range(DC):
                    nc.tensor.matmul(yt_ps[:], a1[dc][:, ts(i, P)], xt[dc][:],
                                     start=(dc == 0), stop=(dc == DC - 1))
                nc.vector.tensor_mul(out=wyt[i][:], in0=yt_ps[:], in1=repr_sb[i][:])
                nc.tensor.matmul(ht_ps[:], qred[:], wyt[i][:],
                                 start=(i == 0), stop=(i == NRT - 1))
            ht_sb = constp.tile([R, T], F32R, tag="ht")
            nc.any.tensor_copy(ht_sb[:], ht_ps[:])

            # ---- CT = RepH * RepR;  deltaT = (alpha*A2)^T @ CT;  xf = x + deltaT ----
            ct = [mid.tile([P, T], F32R, tag=f"mid{i}", name=f"ct{i}") for i in range(NRT)]
            for i in range(NRT):
                rh_ps = psum.tile([P, T], F32, tag="ps")
                nc.tensor.matmul(rh_ps[:], trep[:], ht_sb[:], start=True, stop=True)
                nc.vector.tensor_mul(out=ct[i][:], in0=rh_ps[:], in1=repr_sb[i][:])
            xf = [constp.tile([P, T], F32R, tag=f"xf{i}", name=f"xf{i}") for i in range(DC)]
            for dt in range(DT):
                dl_ps = psum.tile([P, T], F32, tag="ps")
                for i in range(NRT):
                    nc.tensor.matmul(dl_ps[:], a2[i][:, ts(dt, P)], ct[i][:],
                                     start=(i == 0), stop=(i == NRT - 1))
                nc.vector.tensor_add(out=xf[dt][:], in0=dl_ps[:], in1=xt[dt][:])

            # ---- FFN up + exact gelu ----
            g = constp.tile([P, FT, T], F32R, tag="g")
            for ft in range(FT):
                wu = stream.tile([P, DC, P], F32R, tag="wu")
                nc.sync.dma_start(wu[:], wu_d[ft])
                u_ps = psum.tile([P, T], F32, tag="ps")
                for dc in range(DC):
                    nc.tensor.matmul(u_ps[:], wu[:, dc, :], xf[dc][:],
                                     start=(dc == 0), stop=(dc == DC - 1))
                nc.scalar.activation(g[:, ft, :], u_ps[:], AF.Gelu,
                                     bias=bu[:, ft:ft + 1], scale=1.0)

            # ---- FFN down + bias ----
            for dt in range(DT):
                o_ps = psum.tile([P, T], F32, tag="ps")
                wd = None
                for fc in range(FT):
                    if fc % 8 == 0:
                        wd = wdstream.tile([P, 8, P], F32R, tag="wd")
                        nc.sync.dma_start(wd[:], wd_d[dt, :, fc:fc + 8, :])
                    nc.tensor.matmul(o_ps[:], wd[:, fc % 8, :], g[:, fc, :],
                                     start=(fc == 0), stop=(fc == FT - 1))
                ot = stream.tile([P, T], F32, tag="ot")
                nc.scalar.activation(ot[:], o_ps[:], AF.Identity,
                                     bias=bd[:, dt:dt + 1], scale=1.0)
                nc.sync.dma_start(out_d[:, dt, :], ot[:])

    nc.finalize()
    return nc


def _get_nc():
    if _BUILT[0] is None:
        _BUILT[0] = _build_nc()
    return _BUILT[0]


def kernel(x, neuron_idx, neuron_weights, neuron_recipe, basis_A,
           w_up_w, w_up_b, w_down_w, w_down_b, alpha):
    nc = _get_nc()

    x = np.asarray(x, dtype=np.float32).reshape(NCORES * T, D)
    idxf = np.asarray(neuron_idx).astype(np.float32).reshape(NCORES * T, K)
    wgt = np.asarray(neuron_weights, dtype=np.float32).reshape(NCORES * T, K)
    rec = np.asarray(neuron_recipe, dtype=np.float32)
    bA = np.asarray(basis_A, dtype=np.float32)
    wu = np.asarray(w_up_w, dtype=np.float32)
    bu_in = np.asarray(w_up_b, dtype=np.float32)
    wd = np.asarray(w_down_w, dtype=np.float32)
    bd_in = np.asarray(w_down_b, dtype=np.float32)
    alpha_f = float(np.asarray(alpha, dtype=np.float32))

    # replicated operands, packed into the on-device layouts
    a1 = np.ascontiguousarray(
        bA.transpose(1, 0, 2).reshape(D, NB * R)
        .reshape(DC, P, NB * R).transpose(1, 0, 2))
    a2 = np.ascontiguousarray(
        bA.transpose(0, 2, 1).reshape(NB * R, D)
        .reshape(NRT, P, D).transpose(1, 0, 2))
    wu_p = np.ascontiguousarray(wu.reshape(DC, P, FT, P).transpose(2, 1, 0, 3))
    wd_p = np.ascontiguousarray(wd.reshape(FT, P, DT, P).transpose(2, 1, 0, 3))
    bu_t = np.ascontiguousarray(bu_in.reshape(FT, P).T)
    bd_t = np.ascontiguousarray(bd_in.reshape(DT, P).T)
    alpha_c = np.full((P, 1), alpha_f, dtype=np.float32)

    # SEL[n, i, m] = 1 iff n in [4i, 4i+4) and m // 32 == n - 4i
    selpat = np.zeros((NB, NRT, P), dtype=np.float32)
    for n in range(NB):
        i = n // 4
        nloc = n % 4
        selpat[n, i, nloc * R:(nloc + 1) * R] = 1.0
    trep = (np.arange(P)[None, :] % R == np.arange(R)[:, None]).astype(np.float32)
    qred = (np.arange(P)[:, None] % R == np.arange(R)[None, :]).astype(np.float32)
    iota64 = np.broadcast_to(
        np.arange(NN, dtype=np.float32), (P, NN)).copy()
    ident = np.eye(P, dtype=np.float32)

    shared = {
        "recipe": rec, "a1": a1, "a2": a2, "wu": wu_p, "wd": wd_p,
        "bu": bu_t, "bd": bd_t, "alphac": alpha_c, "selpat": selpat,
        "trep": trep, "qred": qred, "iota64": iota64, "ident": ident,
    }
    in_maps = []
    for c in range(NCORES):
        xc = x[c * T:(c + 1) * T]  # [T, D]
        xt = np.ascontiguousarray(xc.T.reshape(DC, P, T).transpose(1, 0, 2))
        in_maps.append({
            "xt": xt,
            "idxf": np.ascontiguousarray(idxf[c * T:(c + 1) * T]),
            "wgt": np.ascontiguousarray(wgt[c * T:(c + 1) * T]),
            **shared,
        })

    res = run_bass_kernel_spmd(nc, in_maps, core_ids=list(range(NCORES)))

    out = np.empty((NCORES * T, D), dtype=np.float32)
    for c in range(NCORES):
        ot = res.results[c]["outT"]  # [P, DT, T]
        out[c * T:(c + 1) * T] = ot.transpose(1, 0, 2).reshape(D, T).T
    return out.reshape(2, 2048, D)
